# revision 35
# baseline (speedup 1.0000x reference)
"""Trainium2 Bass kernel: out = 2 * cummax_W(cummax_H(x)) for x [16,256,128,128] f32.

Precision: gate is rel_err < 2e-2. The device works on xb = bf16(2*x) (host
downcast; the x2 is folded into the input -- exact, since max commutes with
positive scaling and bf16*2 is exact). The only error is the input rounding
(~2^-9 relative; measured 3e-3 against the fp32 reference).

Architecture. The DVE scan (tensor_tensor_scan) runs at a fixed 2 cyc/elem
(any dtype, any FD, any op pair -- the recurrence is the limit) and is the
kernel bottleneck; TT-max on aligned contiguous bf16 runs at 0.5 cyc/elem
(2x_1P mode). BOTH cummax passes therefore use a pair-trick that scans only
half the elements:
  combine: m_k = max(y_2k, y_2k+1)             (TT-max, 0.5 cyc/elem)
  scan:    Z = segmented cummax of m           (2 cyc per PAIR)
  fix:     R_2k = max(Z_{k-1}, y_2k)           (TT-max, 0.5 cyc/elem)
  odd outputs: R_2k+1 = Z_k  (consumed in place, never copied)
Every combine/fix AP keeps the DVE 2x_1P mode: even strides, 4B-aligned
starts, contiguous innermost runs. Each pair scan writes at +1 into a
66-per-slice padded m layout (2 NEG pads per slice) so the fix's shifted
window starts on an even element and reads NEG at slice boundaries.

Engine placement (the load-bearing choices):
  - W pass: the host pre-splits W into even|odd blocks so the combine reads
    two contiguous blocks; the fix writes even results straight into the
    merged PE-transpose input tile, and the odd results (the Z run) are
    merged next to them by a SCALAR-engine copy. (A gpsimd-DMA merge loses:
    SWDGE descriptor rings live in SBUF and their traffic knocks the DVE out
    of 2-port mode; half-width PE transposes also lose: transpose-mode MM
    cost is set by the 128 streamed identity columns, so two [128,64]
    transposes cost twice one [128,128].)
  - PE: full 128x128 bf16 transposes into PSUM (bf16 stays bf16 with
    is_transpose), 8 slices per bank tile.
  - H pass: the scalar engine deinterleaves h-even/h-odd while staging
    PSUM->SBUF (strided reads there are off the DVE critical path).
  - Stores: h-even results and h-odd (Z) results go out as two DMA streams
    (scalar HWDGE + gpsimd SWDGE) into a [w', s, hE|hO] DRAM layout; the
    host re-interleaves h, inverse-permutes w', and upcasts.

Per core (batch-parallel: 2 batches = 512 (b,c) slices), supertiles of g=16
slices with 8-slice taper chunks at both ends for pipeline fill/drain.
"""

from contextlib import ExitStack

import numpy as np

import concourse.bass as bass
import concourse.tile as tile
from concourse import bacc, mybir
from concourse.bass_utils import run_bass_kernel_spmd

N_CORES = 8
B, C, H, W = 16, 256, 128, 128
S = (B // N_CORES) * C
NEG = -3.0e38

BF16 = mybir.dt.bfloat16

LAST_RESULTS = None


def build_nc(n_slices: int = S, g: int = 16, bufs: int = 8) -> bass.Bass:
    nc = bacc.Bacc(None, target_bir_lowering=False)
    # h-major input with W deinterleaved: x[h, s*W + (wE|wO)]
    # output o[w', s*128 + (hE|hO)], w' = (evens | odds)
    x = nc.declare_dram_parameter("x", [H, n_slices * W], BF16, isOutput=False)
    cst = nc.declare_dram_parameter("cst", [128, 128 + g * 66], BF16, isOutput=False)
    o = nc.declare_dram_parameter("o", [W, n_slices * H], BF16, isOutput=True)

    head = [8, 8]
    tail_t = [8, 4, 4]
    chunks = []
    pos = 0
    for c in head:
        chunks.append((pos, c))
        pos += c
    tail = n_slices - sum(tail_t)
    while pos < tail:
        chunks.append((pos, g))
        pos += g
    for c in tail_t:
        chunks.append((pos, c))
        pos += c
    assert pos == n_slices, (pos, n_slices)

    with ExitStack() as ctx:
        tc = ctx.enter_context(tile.TileContext(nc))
        consts = ctx.enter_context(tc.tile_pool(name="consts", bufs=1))
        cstt = consts.tile([128, 128 + g * 66], BF16)
        nc.scalar.dma_start(out=cstt[:], in_=cst.ap())
        ident = cstt[:, 0:128]
        bias_m = cstt[:, 128 : 128 + g * 66]

        xpool = ctx.enter_context(tc.tile_pool(name="xt", bufs=bufs))
        apool = ctx.enter_context(tc.tile_pool(name="at", bufs=bufs))
        epool = ctx.enter_context(tc.tile_pool(name="be", bufs=bufs))
        opool = ctx.enter_context(tc.tile_pool(name="bo", bufs=bufs))
        mpool = ctx.enter_context(tc.tile_pool(name="mt", bufs=4))
        zwpool = ctx.enter_context(tc.tile_pool(name="zw", bufs=bufs))
        zhpool = ctx.enter_context(tc.tile_pool(name="zh", bufs=bufs))
        rpool = ctx.enter_context(tc.tile_pool(name="rt", bufs=bufs))
        pa_pool = ctx.enter_context(tc.tile_pool(name="pa", bufs=6, space="PSUM"))

        xv = x.ap()
        ov = o.ap()

        for ci, (s0, gc) in enumerate(chunks):
            fw = gc * W
            hw = gc * 64
            xt = xpool.tile([128, fw], BF16, tag="xt")
            nc.sync.dma_start(out=xt[:], in_=xv[:, s0 * W : s0 * W + fw])
            xts = xt[:].rearrange("p (s e) -> p s e", s=gc)

            # --- W pass (pair trick) ---
            mtw = mpool.tile([128, g * 66], BF16, tag="mtw")
            mtwv = mtw[:, : gc * 66].rearrange("p (s e) -> p s e", s=gc)
            if ci < 4:
                mf = mtw[:].rearrange("p (s e) -> p s e", s=g)
                nc.gpsimd.memset(mf[:, :, 0:2], NEG)
            nc.vector.tensor_tensor(
                mtwv[:, :, 2:66], xts[:, :, 0:64], xts[:, :, 64:128],
                mybir.AluOpType.max,
            )
            zw = zwpool.tile([128, gc * 66 + 4], BF16, tag="zw")
            nc.vector.tensor_tensor_scan(
                zw[:, 1 : gc * 66 + 1], bias_m[:, : gc * 66], mtw[:, : gc * 66],
                0.0, mybir.AluOpType.add, mybir.AluOpType.max,
            )
            zwv = zw[:, : gc * 66].rearrange("p (s e) -> p s e", s=gc)
            at = apool.tile([128, fw], BF16, tag="at")
            ats = at[:].rearrange("p (s e) -> p s e", s=gc)
            nc.vector.tensor_tensor(
                ats[:, :, 0:64], zwv[:, :, 2:66], xts[:, :, 0:64],
                mybir.AluOpType.max,
            )
            zwz = zw[:, 3 : 3 + gc * 66].rearrange("p (s e) -> p s e", s=gc)
            nc.scalar.copy(ats[:, :, 64:128], zwz[:, :, 0:64])

            # --- transpose + deinterleaved scalar staging ---
            btE = epool.tile([128, hw], BF16, tag="be")
            btO = opool.tile([128, hw], BF16, tag="bo")
            btEv = btE[:].rearrange("p (s e) -> p s e", s=gc)
            btOv = btO[:].rearrange("p (s e) -> p s e", s=gc)
            nb = (gc + 7) // 8
            sl = gc // nb
            for hb in range(nb):
                pa = pa_pool.tile([128, sl * 128], BF16, tag="pa")
                for j in range(sl):
                    s = hb * sl + j
                    nc.tensor.transpose(
                        pa[:, j * 128 : (j + 1) * 128],
                        at[:, s * 128 : (s + 1) * 128],
                        ident[:],
                    )
                pav = pa[:].rearrange("p (s hj hb) -> p s hj hb", s=sl, hb=2)
                nc.scalar.copy(btEv[:, hb * sl : (hb + 1) * sl], pav[:, :, :, 0])
                nc.scalar.copy(btOv[:, hb * sl : (hb + 1) * sl], pav[:, :, :, 1])

            # --- H pass (pair trick) ---
            mth = mpool.tile([128, g * 66], BF16, tag="mth")
            mthv = mth[:, : gc * 66].rearrange("p (s e) -> p s e", s=gc)
            if ci < 4:
                mf = mth[:].rearrange("p (s e) -> p s e", s=g)
                nc.gpsimd.memset(mf[:, :, 0:2], NEG)
            nc.vector.tensor_tensor(
                mthv[:, :, 2:66], btEv[:], btOv[:], mybir.AluOpType.max
            )
            zh = zhpool.tile([128, gc * 66 + 4], BF16, tag="zh")
            nc.vector.tensor_tensor_scan(
                zh[:, 1 : gc * 66 + 1], bias_m[:, : gc * 66], mth[:, : gc * 66],
                0.0, mybir.AluOpType.add, mybir.AluOpType.max,
            )
            zhv = zh[:, : gc * 66].rearrange("p (s e) -> p s e", s=gc)
            rt = rpool.tile([128, hw], BF16, tag="rt")
            rts = rt[:].rearrange("p (s e) -> p s e", s=gc)
            nc.vector.tensor_tensor(
                rts[:], zhv[:, :, 2:66], btEv[:], mybir.AluOpType.max
            )
            ovv = ov[:, s0 * H : s0 * H + fw].rearrange("p (s e) -> p s e", s=gc)
            nc.scalar.dma_start(out=ovv[:, :, 0:64], in_=rts[:])
            zhz = zh[:, 3 : 3 + gc * 66].rearrange("p (s e) -> p s e", s=gc)
            nc.gpsimd.dma_start(out=ovv[:, :, 64:128], in_=zhz[:, :, 0:64])
    nc.finalize()
    return nc


def kernel(x: np.ndarray) -> np.ndarray:
    global LAST_RESULTS
    import ml_dtypes

    assert x.shape == (B, C, H, W)
    xb = (np.asarray(x, dtype=np.float32) * 2.0).astype(ml_dtypes.bfloat16)
    xs = xb.reshape(N_CORES, S, H, W)
    xd = np.concatenate([xs[..., 0::2], xs[..., 1::2]], axis=-1)
    g = 16
    cst = np.zeros((128, 128 + g * 66), dtype=ml_dtypes.bfloat16)
    cst[:, 0:128] = np.eye(128, dtype=np.float32).astype(ml_dtypes.bfloat16)
    bias = np.zeros((128, g * 66), dtype=np.float32)
    bias[:, 0 : g * 66 : 66] = NEG
    cst[:, 128:] = bias.astype(ml_dtypes.bfloat16)
    in_maps = [
        {
            "x": np.ascontiguousarray(xd[i].transpose(1, 0, 2)).reshape(H, S * W),
            "cst": cst,
        }
        for i in range(N_CORES)
    ]
    nc = build_nc(S, g=16, bufs=8)
    res = run_bass_kernel_spmd(nc, in_maps, core_ids=list(range(N_CORES)))
    LAST_RESULTS = res
    out = np.empty((N_CORES, S, H, W), dtype=np.float32)
    for i in range(N_CORES):
        oi = np.asarray(res.results[i]["o"]).reshape(W, S, 2, 64).astype(np.float32)
        t = np.empty((S, H, W), dtype=np.float32)
        t[:, 0::2, 0::2] = oi[0:64, :, 0, :].transpose(1, 2, 0)
        t[:, 1::2, 0::2] = oi[0:64, :, 1, :].transpose(1, 2, 0)
        t[:, 0::2, 1::2] = oi[64:128, :, 0, :].transpose(1, 2, 0)
        t[:, 1::2, 1::2] = oi[64:128, :, 1, :].transpose(1, 2, 0)
        out[i] = t
    return out.reshape(B, C, H, W)


# revision 36
# speedup vs baseline: 1.0144x; 1.0144x over previous
"""Trainium2 Bass kernel: out = 2 * cummax_W(cummax_H(x)) for x [16,256,128,128] f32.

Precision: gate is rel_err < 2e-2. The device works on xb = bf16(2*x) (host
downcast; the x2 is folded into the input -- exact, since max commutes with
positive scaling and bf16*2 is exact). The only error is the input rounding
(~2^-9 relative; measured 3e-3 against the fp32 reference).

Architecture. The DVE scan (tensor_tensor_scan) runs at a fixed 2 cyc/elem
(any dtype, any FD, any op pair -- the recurrence is the limit) and is the
kernel bottleneck; TT-max on aligned contiguous bf16 runs at 0.5 cyc/elem
(2x_1P mode). BOTH cummax passes therefore use a pair-trick that scans only
half the elements:
  combine: m_k = max(y_2k, y_2k+1)             (TT-max, 0.5 cyc/elem)
  scan:    Z = segmented cummax of m           (2 cyc per PAIR)
  fix:     R_2k = max(Z_{k-1}, y_2k)           (TT-max, 0.5 cyc/elem)
  odd outputs: R_2k+1 = Z_k  (consumed in place, never copied)
Every combine/fix AP keeps the DVE 2x_1P mode: even strides, 4B-aligned
starts, contiguous innermost runs. Each pair scan writes at +1 into a
66-per-slice padded m layout (2 NEG pads per slice) so the fix's shifted
window starts on an even element and reads NEG at slice boundaries.

Engine placement (the load-bearing choices):
  - W pass: the host pre-splits W into even|odd blocks so the combine reads
    two contiguous blocks; the fix writes even results straight into the
    merged PE-transpose input tile, and the odd results (the Z run) are
    merged next to them by a SCALAR-engine copy. (A gpsimd-DMA merge loses:
    SWDGE descriptor rings live in SBUF and their traffic knocks the DVE out
    of 2-port mode; half-width PE transposes also lose: transpose-mode MM
    cost is set by the 128 streamed identity columns, so two [128,64]
    transposes cost twice one [128,128].)
  - PE: full 128x128 bf16 transposes into PSUM (bf16 stays bf16 with
    is_transpose), 8 slices per bank tile.
  - H pass: the scalar engine deinterleaves h-even/h-odd while staging
    PSUM->SBUF (strided reads there are off the DVE critical path).
  - Stores: h-even results and h-odd (Z) results go out as two DMA streams
    (scalar HWDGE + gpsimd SWDGE) into a [w', s, hE|hO] DRAM layout; the
    host re-interleaves h, inverse-permutes w', and upcasts.

Per core (batch-parallel: 2 batches = 512 (b,c) slices), supertiles of g=16
slices with 8-slice taper chunks at both ends for pipeline fill/drain.
"""

from contextlib import ExitStack

import numpy as np

import concourse.bass as bass
import concourse.tile as tile
from concourse import bacc, mybir
from concourse.bass_utils import run_bass_kernel_spmd

N_CORES = 8
B, C, H, W = 16, 256, 128, 128
S = (B // N_CORES) * C
NEG = -3.0e38

BF16 = mybir.dt.bfloat16

LAST_RESULTS = None


def build_nc(n_slices: int = S, g: int = 16, bufs: int = 8) -> bass.Bass:
    nc = bacc.Bacc(None, target_bir_lowering=False)
    # h-major input with W deinterleaved: x[h, s*W + (wE|wO)]
    # output o[w', s*128 + (hE|hO)], w' = (evens | odds)
    x = nc.declare_dram_parameter("x", [H, n_slices * W], BF16, isOutput=False)
    cst = nc.declare_dram_parameter("cst", [128, 128 + g * 66], BF16, isOutput=False)
    o = nc.declare_dram_parameter("o", [W, n_slices * H], BF16, isOutput=True)

    head = [8, 8]
    tail_t = [8, 4, 4]
    chunks = []
    pos = 0
    for c in head:
        chunks.append((pos, c))
        pos += c
    tail = n_slices - sum(tail_t)
    while pos < tail:
        chunks.append((pos, g))
        pos += g
    for c in tail_t:
        chunks.append((pos, c))
        pos += c
    assert pos == n_slices, (pos, n_slices)

    with ExitStack() as ctx:
        tc = ctx.enter_context(tile.TileContext(nc))
        consts = ctx.enter_context(tc.tile_pool(name="consts", bufs=1))
        cstt = consts.tile([128, 128 + g * 66], BF16)
        nc.scalar.dma_start(out=cstt[:], in_=cst.ap())
        ident = cstt[:, 0:128]
        bias_m = cstt[:, 128 : 128 + g * 66]

        xpool = ctx.enter_context(tc.tile_pool(name="xt", bufs=bufs))
        apool = ctx.enter_context(tc.tile_pool(name="at", bufs=bufs))
        epool = ctx.enter_context(tc.tile_pool(name="be", bufs=bufs))
        opool = ctx.enter_context(tc.tile_pool(name="bo", bufs=bufs))
        mpool = ctx.enter_context(tc.tile_pool(name="mt", bufs=4))
        zwpool = ctx.enter_context(tc.tile_pool(name="zw", bufs=bufs))
        zhpool = ctx.enter_context(tc.tile_pool(name="zh", bufs=bufs))
        rpool = ctx.enter_context(tc.tile_pool(name="rt", bufs=bufs))
        pa_pool = ctx.enter_context(tc.tile_pool(name="pa", bufs=6, space="PSUM"))

        xv = x.ap()
        ov = o.ap()

        def emit_w(ci, s0, gc):
            fw = gc * W
            hw = gc * 64
            xt = xpool.tile([128, fw], BF16, tag="xt")
            nc.sync.dma_start(out=xt[:], in_=xv[:, s0 * W : s0 * W + fw])
            xts = xt[:].rearrange("p (s e) -> p s e", s=gc)

            # --- W pass (pair trick) ---
            mtw = mpool.tile([128, g * 66], BF16, tag="mtw")
            mtwv = mtw[:, : gc * 66].rearrange("p (s e) -> p s e", s=gc)
            if ci < 4:
                mf = mtw[:].rearrange("p (s e) -> p s e", s=g)
                nc.gpsimd.memset(mf[:, :, 0:2], NEG)
            nc.vector.tensor_tensor(
                mtwv[:, :, 2:66], xts[:, :, 0:64], xts[:, :, 64:128],
                mybir.AluOpType.max,
            )
            zw = zwpool.tile([128, gc * 66 + 4], BF16, tag="zw")
            nc.vector.tensor_tensor_scan(
                zw[:, 1 : gc * 66 + 1], bias_m[:, : gc * 66], mtw[:, : gc * 66],
                0.0, mybir.AluOpType.add, mybir.AluOpType.max,
            )
            zwv = zw[:, : gc * 66].rearrange("p (s e) -> p s e", s=gc)
            at = apool.tile([128, fw], BF16, tag="at")
            ats = at[:].rearrange("p (s e) -> p s e", s=gc)
            nc.vector.tensor_tensor(
                ats[:, :, 0:64], zwv[:, :, 2:66], xts[:, :, 0:64],
                mybir.AluOpType.max,
            )
            zwz = zw[:, 3 : 3 + gc * 66].rearrange("p (s e) -> p s e", s=gc)
            nc.scalar.copy(ats[:, :, 64:128], zwz[:, :, 0:64])

            # --- transpose + deinterleaved scalar staging ---
            btE = epool.tile([128, hw], BF16, tag="be")
            btO = opool.tile([128, hw], BF16, tag="bo")
            btEv = btE[:].rearrange("p (s e) -> p s e", s=gc)
            btOv = btO[:].rearrange("p (s e) -> p s e", s=gc)
            nb = (gc + 7) // 8
            sl = gc // nb
            for hb in range(nb):
                pa = pa_pool.tile([128, sl * 128], BF16, tag="pa")
                for j in range(sl):
                    sx = hb * sl + j
                    nc.tensor.transpose(
                        pa[:, j * 128 : (j + 1) * 128],
                        at[:, sx * 128 : (sx + 1) * 128],
                        ident[:],
                    )
                pav = pa[:].rearrange("p (s hj hb) -> p s hj hb", s=sl, hb=2)
                nc.scalar.copy(btEv[:, hb * sl : (hb + 1) * sl], pav[:, :, :, 0])
                nc.scalar.copy(btOv[:, hb * sl : (hb + 1) * sl], pav[:, :, :, 1])
            return ci, s0, gc, btE, btO

        def emit_h(st):
            ci, s0, gc, btE, btO = st
            fw = gc * W
            hw = gc * 64
            btEv = btE[:].rearrange("p (s e) -> p s e", s=gc)
            btOv = btO[:].rearrange("p (s e) -> p s e", s=gc)
            mth = mpool.tile([128, g * 66], BF16, tag="mth")
            mthv = mth[:, : gc * 66].rearrange("p (s e) -> p s e", s=gc)
            if ci < 4:
                mf = mth[:].rearrange("p (s e) -> p s e", s=g)
                nc.gpsimd.memset(mf[:, :, 0:2], NEG)
            nc.vector.tensor_tensor(
                mthv[:, :, 2:66], btEv[:], btOv[:], mybir.AluOpType.max
            )
            zh = zhpool.tile([128, gc * 66 + 4], BF16, tag="zh")
            nc.vector.tensor_tensor_scan(
                zh[:, 1 : gc * 66 + 1], bias_m[:, : gc * 66], mth[:, : gc * 66],
                0.0, mybir.AluOpType.add, mybir.AluOpType.max,
            )
            zhv = zh[:, : gc * 66].rearrange("p (s e) -> p s e", s=gc)
            rt = rpool.tile([128, hw], BF16, tag="rt")
            rts = rt[:].rearrange("p (s e) -> p s e", s=gc)
            nc.vector.tensor_tensor(
                rts[:], zhv[:, :, 2:66], btEv[:], mybir.AluOpType.max
            )
            ovv = ov[:, s0 * H : s0 * H + fw].rearrange("p (s e) -> p s e", s=gc)
            nc.scalar.dma_start(out=ovv[:, :, 0:64], in_=rts[:])
            zhz = zh[:, 3 : 3 + gc * 66].rearrange("p (s e) -> p s e", s=gc)
            nc.gpsimd.dma_start(out=ovv[:, :, 64:128], in_=zhz[:, :, 0:64])

        # software-pipelined emission: W of chunk i+1 precedes H of chunk i,
        # so the in-order DVE never head-of-line blocks on PE/scalar staging
        pend = None
        for ci, (s0, gc) in enumerate(chunks):
            st = emit_w(ci, s0, gc)
            if pend is not None:
                emit_h(pend)
            pend = st
        emit_h(pend)
    nc.finalize()
    return nc


def kernel(x: np.ndarray) -> np.ndarray:
    global LAST_RESULTS
    import ml_dtypes

    assert x.shape == (B, C, H, W)
    xb = (np.asarray(x, dtype=np.float32) * 2.0).astype(ml_dtypes.bfloat16)
    xs = xb.reshape(N_CORES, S, H, W)
    xd = np.concatenate([xs[..., 0::2], xs[..., 1::2]], axis=-1)
    g = 16
    cst = np.zeros((128, 128 + g * 66), dtype=ml_dtypes.bfloat16)
    cst[:, 0:128] = np.eye(128, dtype=np.float32).astype(ml_dtypes.bfloat16)
    bias = np.zeros((128, g * 66), dtype=np.float32)
    bias[:, 0 : g * 66 : 66] = NEG
    cst[:, 128:] = bias.astype(ml_dtypes.bfloat16)
    in_maps = [
        {
            "x": np.ascontiguousarray(xd[i].transpose(1, 0, 2)).reshape(H, S * W),
            "cst": cst,
        }
        for i in range(N_CORES)
    ]
    nc = build_nc(S, g=16, bufs=8)
    res = run_bass_kernel_spmd(nc, in_maps, core_ids=list(range(N_CORES)))
    LAST_RESULTS = res
    out = np.empty((N_CORES, S, H, W), dtype=np.float32)
    for i in range(N_CORES):
        oi = np.asarray(res.results[i]["o"]).reshape(W, S, 2, 64).astype(np.float32)
        t = np.empty((S, H, W), dtype=np.float32)
        t[:, 0::2, 0::2] = oi[0:64, :, 0, :].transpose(1, 2, 0)
        t[:, 1::2, 0::2] = oi[0:64, :, 1, :].transpose(1, 2, 0)
        t[:, 0::2, 1::2] = oi[64:128, :, 0, :].transpose(1, 2, 0)
        t[:, 1::2, 1::2] = oi[64:128, :, 1, :].transpose(1, 2, 0)
        out[i] = t
    return out.reshape(B, C, H, W)
